# revision 21
# baseline (speedup 1.0000x reference)
"""GQA causal attention (RoPE, B=2 S=2048 D=2048 H=16 KV=8 HD=128) on 8 trn2 cores.

Strategy: head-parallel. Each core c owns q-heads {2c, 2c+1} and kv-head c.
Host replicates x (pre-transposed to [D, B*S], bf16) to all cores; all
projections, RoPE and causal attention are head-sharded (zero comm). Two
AllToAlls (one per local q-head, 1 MB/rank each, bf16) convert the attention
output from head-sharding to row-sharding overlapped with the other head's
attention, then each core computes its 512-row slice of the output projection
with the full Wo. Host concatenates the 8 row shards.

Layout trick: everything is computed transposed (qT/kT = [HD, seq] with HD on
partitions, scores as [k, q]) so no on-device activation transposes are
needed; the only transposes are 128x128 PE transposes of vT -> v. Softmax runs
max-free (scores are small by construction), the denominator comes from a
ones-vector matmul on the PE, and the causal mask is added in PSUM via an
identity-matmul of a host-provided mask tile. All matmuls run bf16 (1 cyc/row
on the PE; fp32 accumulates in PSUM).
"""

import os
import sys

import numpy as np

if "/opt/trn_rl_repo" not in sys.path:
    sys.path.insert(0, "/opt/trn_rl_repo")

CORES = 8


def build_nc(B, S, D, H, KV, HD, HO, QC):
    """Build the SPMD bass graph (same graph for all 8 cores)."""
    import concourse.bacc as bacc
    import concourse.tile as tile
    from concourse import mybir
    from contextlib import ExitStack

    f32 = mybir.dt.float32
    bf16 = mybir.dt.bfloat16
    ACT = mybir.ActivationFunctionType

    QH = H // CORES               # q heads per core (2)
    R = B * S                     # total rows (4096)
    RO = R // CORES               # output rows per core (512) == QC
    assert QC == RO
    DK = D // 128                 # k-tiles over model dim (16)
    RC = 512                      # row-chunk width for projections
    NCH = R // RC                 # projection row chunks (8)
    NQC = S // QC                 # q chunks per batch (4)
    NKT = S // 128                # k tiles per batch (16)
    NT = QC // 128                # diagonal mask patterns (4)
    NRT = RO // 128               # out row tiles per core (4)
    OC = min(D, 512)              # out col chunk
    NOC = D // OC                 # out col chunks (4)
    HG = H                        # total heads in O-proj
    scale = float(HD) ** -0.5

    nc = bacc.Bacc("TRN2", target_bir_lowering=False, debug=False,
                   num_devices=CORES)

    xT = nc.dram_tensor("xT", [D, R], bf16, kind="ExternalInput")
    cosT = nc.dram_tensor("cosT", [HD, S], bf16, kind="ExternalInput")
    sinT = nc.dram_tensor("sinT", [HD, S], bf16, kind="ExternalInput")
    wq = nc.dram_tensor("wq", [D, QH * HD], bf16, kind="ExternalInput")
    wk = nc.dram_tensor("wk", [D, HD], bf16, kind="ExternalInput")
    wv = nc.dram_tensor("wv", [D, HD], bf16, kind="ExternalInput")
    wo = nc.dram_tensor("wo", [HO, D], bf16, kind="ExternalInput")
    masks = nc.dram_tensor("masks", [128, NT * QC], bf16, kind="ExternalInput")
    ident = nc.dram_tensor("ident", [128, 128], bf16, kind="ExternalInput")
    ones = nc.dram_tensor("ones", [128, 128], bf16, kind="ExternalInput")
    out = nc.dram_tensor("out", [RO, D], f32, kind="ExternalOutput")

    with tile.TileContext(nc) as tc, ExitStack() as top:
        dram = top.enter_context(tc.tile_pool(name="dram", bufs=1, space="DRAM"))
        consts = top.enter_context(tc.tile_pool(name="consts", bufs=1))
        resid = top.enter_context(tc.tile_pool(name="resid", bufs=1))

        a2a_in = dram.tile([CORES * QH, 128, QC], bf16)
        a2a_out = dram.tile([CORES * QH, 128, QC], bf16)

        ident_sb = consts.tile([128, 128], bf16)
        ones_sb = consts.tile([128, 128], bf16)
        mask_sb = consts.tile([128, NT * QC], bf16)

        # residents produced by projection phase, consumed by attention
        qT_sb = resid.tile([128, QH, R], bf16)       # [hd, head, row]
        kT_sb = resid.tile([128, R], bf16)           # [hd, row]
        v_sb = resid.tile([128, R], bf16)            # [kpos%128, ktile*HD+hd]

        # full Wo resident (prefetch recorded mid-projection; no deps)
        wo_all = resid.tile([128, HG, D], bf16)
        attn_all = resid.tile([128, HG, QC], bf16)

        # ------------------------------- phase 1: projections + rope
        with ExitStack() as ph1:
            ropec = ph1.enter_context(tc.tile_pool(name="ropec", bufs=1))
            wpool = ph1.enter_context(tc.tile_pool(name="wpool", bufs=1))
            xpool = ph1.enter_context(tc.tile_pool(name="xpool", bufs=5))
            rtmp = ph1.enter_context(tc.tile_pool(name="rtmp", bufs=2))
            psA = ph1.enter_context(tc.tile_pool(name="psA", bufs=2, space="PSUM"))
            psTR = ph1.enter_context(tc.tile_pool(name="psTR", bufs=2, space="PSUM"))

            wq_sb = wpool.tile([128, DK, QH * HD], bf16)
            wk_sb = wpool.tile([128, DK, HD], bf16)
            wv_sb = wpool.tile([128, DK, HD], bf16)
            cos_sb = ropec.tile([128, S], bf16)
            sin_sb = ropec.tile([128, S], bf16)
            wq_r = wq.ap().rearrange("(kt p) c -> p kt c", p=128)
            wk_r = wk.ap().rearrange("(kt p) c -> p kt c", p=128)
            wv_r = wv.ap().rearrange("(kt p) c -> p kt c", p=128)

            # chunk-0 x + weights interleaved kt-granular so MM kt can start
            # as soon as its own pieces land (subtile deps)
            DKH = DK // 2
            xch0 = [xpool.tile([128, DKH, RC], bf16, tag="xch", name="xch0")
                    for _ in range(2)]
            xsrc0 = xT[:, 0:RC].rearrange("(kt p) c -> p kt c", p=128)
            ncs = S // 512
            for kt in range(DK):
                nc.sync.dma_start(out=xch0[kt // DKH][:, kt % DKH, :],
                                  in_=xsrc0[:, kt, :])
                nc.sync.dma_start(out=wq_sb[:, kt, :], in_=wq_r[:, kt, :])
                nc.sync.dma_start(out=wk_sb[:, kt, :], in_=wk_r[:, kt, :])
                nc.sync.dma_start(out=wv_sb[:, kt, :], in_=wv_r[:, kt, :])
                if kt % 4 == 3:
                    cs = kt // 4
                    if cs < ncs:
                        sl = slice(cs * 512, (cs + 1) * 512)
                        nc.sync.dma_start(out=cos_sb[:, sl], in_=cosT[:, sl])
                        nc.sync.dma_start(out=sin_sb[:, sl], in_=sinT[:, sl])
            for cs in range(DK // 4, ncs):
                sl = slice(cs * 512, (cs + 1) * 512)
                nc.sync.dma_start(out=cos_sb[:, sl], in_=cosT[:, sl])
                nc.sync.dma_start(out=sin_sb[:, sl], in_=sinT[:, sl])
            nc.sync.dma_start(out=ident_sb, in_=ident[:, :])
            nc.sync.dma_start(out=ones_sb, in_=ones[:, :])
            nc.sync.dma_start(out=mask_sb, in_=masks[:, :])

            half = HD // 2

            def rope(pp, dst, poff):
                c_sl = cos_sb[:, poff:poff + RC]
                s_sl = sin_sb[:, poff:poff + RC]
                t1 = rtmp.tile([128, RC], f32, tag="t1", name="t1")
                t2 = rtmp.tile([128, RC], f32, tag="t2", name="t2")
                nc.vector.tensor_mul(t1, pp, c_sl)
                nc.vector.tensor_mul(t2[0:half, :], pp[half:128, :], s_sl[0:half, :])
                nc.vector.tensor_mul(t2[half:128, :], pp[0:half, :], s_sl[half:128, :])
                nc.vector.tensor_add(dst, t1, t2)

            assert NCH % 2 == 0
            for p in range(NCH // 2):
                na, nb = 2 * p, 2 * p + 1
                xs = []
                for n in (na, nb):
                    if n == 0:
                        xs.append(xch0)
                    else:
                        xsrc = xT[:, n * RC:(n + 1) * RC].rearrange(
                            "(kt p) c -> p kt c", p=128)
                        halves = []
                        for hh in range(2):
                            xc = xpool.tile([128, DKH, RC], bf16, tag="xch", name="xch")
                            for q4 in range(4):
                                sl = slice(q4 * DKH // 4, (q4 + 1) * DKH // 4)
                                nc.sync.dma_start(
                                    out=xc[:, sl, :],
                                    in_=xsrc[:, hh * DKH + q4 * DKH // 4:
                                             hh * DKH + (q4 + 1) * DKH // 4, :])
                            halves.append(xc)
                        xs.append(halves)
                if p == min(1, NCH // 2 - 1):
                    # prefetch Wo now: pipeline warm, sync queues mostly free
                    wo_r = wo.ap().rearrange("(g p) n -> p g n", p=128)
                    for q8 in range(8):
                        sl = slice(q8 * HG // 8, (q8 + 1) * HG // 8)
                        nc.sync.dma_start(out=wo_all[:, sl, :], in_=wo_r[:, sl, :])

                for oi in range(QH + 2):   # QH q heads, then k, then vT
                    ppab = [psA.tile([128, RC], f32, tag=f"pp{ci}", name=f"pp{ci}")
                            for ci in range(2)]
                    if oi < QH:
                        wcol = (wq_sb, oi * HD)
                    elif oi == QH:
                        wcol = (wk_sb, 0)
                    else:
                        wcol = (wv_sb, 0)
                    for kt in range(DK):
                        wsb = wcol[0][:, kt, wcol[1]:wcol[1] + HD]
                        for ci in range(2):
                            nc.tensor.matmul(
                                ppab[ci], lhsT=wsb,
                                rhs=xs[ci][kt // DKH][:, kt % DKH, :],
                                start=(kt == 0), stop=(kt == DK - 1))
                    for ci, n in enumerate((na, nb)):
                        pp = ppab[ci]
                        poff = (n * RC) % S
                        if oi < QH:
                            rope(pp, qT_sb[:, oi, n * RC:(n + 1) * RC], poff)
                        elif oi == QH:
                            rope(pp, kT_sb[:, n * RC:(n + 1) * RC], poff)
                        else:
                            vt_sb = rtmp.tile([128, RC], bf16, tag="vt", name="vt")
                            nc.scalar.activation(vt_sb, pp, ACT.Copy)
                            for j in range(RC // 128):
                                ptr_ = psTR.tile([128, 128], bf16, tag="ptr", name="ptr")
                                nc.tensor.transpose(ptr_, vt_sb[:, j * 128:(j + 1) * 128], ident_sb)
                                rti = n * (RC // 128) + j
                                nc.scalar.activation(v_sb[:, rti * 128:(rti + 1) * 128], ptr_, ACT.Copy)

        # ------------------------------- phase 2: attention (h-paired)
        with ExitStack() as ph2:
            probs = ph2.enter_context(tc.tile_pool(name="probs", bufs=36))
            atmp = ph2.enter_context(tc.tile_pool(name="atmp", bufs=3))
            dens = ph2.enter_context(tc.tile_pool(name="dens", bufs=2))
            psS = ph2.enter_context(tc.tile_pool(name="psS", bufs=3, space="PSUM"))
            psO = ph2.enter_context(tc.tile_pool(name="psO", bufs=1, space="PSUM"))
            psD = ph2.enter_context(tc.tile_pool(name="psD", bufs=1, space="PSUM"))
            psB = ph2.enter_context(tc.tile_pool(name="psB", bufs=1, space="PSUM"))

            from concourse import mybir as _mb
            for b in range(B):
                for qc in range(NQC - 1, -1, -1):
                    nkt = (qc + 1) * NT
                    po = [psO.tile([128, QC], f32, tag=f"po{h}", name=f"po{h}")
                          for h in range(QH)]
                    pden = [psD.tile([1, QC], f32, tag=f"pden{h}", name=f"pden{h}")
                            for h in range(QH)]
                    prs = {}
                    offs = {}
                    # scores + exp (kT ldweights shared across heads)
                    for kt in range(nkt):
                        dj = kt - qc * NT   # >=0 on diagonal block
                        o = max(dj, 0) * 128   # first valid q col in chunk
                        kl = kT_sb[:, b * S + kt * 128: b * S + (kt + 1) * 128]
                        for h in range(QH):
                            sc = psS.tile([128, QC], f32, tag="sc", name="sc")
                            nc.tensor.matmul(
                                sc[:, o:QC], lhsT=kl,
                                rhs=qT_sb[:, h, b * S + qc * QC + o: b * S + (qc + 1) * QC],
                                start=True, stop=(dj < 0))
                            if dj >= 0:
                                nc.tensor.matmul(
                                    sc[:, o:QC], lhsT=ident_sb,
                                    rhs=mask_sb[:, dj * QC + o:(dj + 1) * QC],
                                    start=False, stop=True)
                            pr = probs.tile([128, QC], bf16, tag="pr", name="pr")
                            nc.scalar.activation(pr[:, o:QC], sc[:, o:QC],
                                                 ACT.Exp, scale=scale)
                            prs[(h, kt)] = pr
                            offs[kt] = o
                    # PV accumulation (v ldweights shared across heads)
                    for kt in range(nkt):
                        ktg = b * NKT + kt
                        o = offs[kt]
                        vl = v_sb[:, ktg * 128:(ktg + 1) * 128]
                        for h in range(QH):
                            nc.tensor.matmul(
                                po[h][:, o:QC], lhsT=vl, rhs=prs[(h, kt)][:, o:QC],
                                start=(kt == 0), stop=(kt == nkt - 1))
                    # denominators (ones ldweights shared across whole batch)
                    for h in range(QH):
                        for kt in range(nkt):
                            o = offs[kt]
                            nc.tensor.matmul(
                                pden[h][:, o:QC], lhsT=ones_sb[:, 0:1],
                                rhs=prs[(h, kt)][:, o:QC],
                                start=(kt == 0), stop=(kt == nkt - 1))
                    # normalize + ship to A2A bounce.  Copy po out of PSUM
                    # first (frees the accumulator bank for the next chunk
                    # without waiting on the reciprocal chain).
                    d = b * NQC + qc   # dest core for these q rows
                    for h in range(QH):
                        at = atmp.tile([128, QC], bf16, tag="at", name="at")
                        nc.scalar.activation(at, po[h], ACT.Copy)
                        den = dens.tile([1, QC], f32, tag="den", name="den")
                        nc.vector.reciprocal_approx_fast(den, pden[h])
                        den_b = dens.tile([1, QC], bf16, tag="denb", name="den_b")
                        nc.scalar.activation(den_b, den, ACT.Copy)
                        pbc = psB.tile([128, QC], f32, tag="pbc", name="pbc")
                        nc.tensor.matmul(pbc, lhsT=ones_sb[0:1, :], rhs=den_b,
                                         start=True, stop=True)
                        bc = atmp.tile([128, QC], bf16, tag="bc", name="bc")
                        nc.scalar.activation(bc, pbc, ACT.Copy)
                        anorm = atmp.tile([128, QC], bf16, tag="an", name="anorm")
                        nc.vector.tensor_mul(anorm, at, bc)
                        nc.sync.dma_start(out=a2a_in[d * QH + h], in_=anorm)

            nc.gpsimd.collective_compute(
                "AllToAll", _mb.AluOpType.bypass,
                ins=[a2a_in.opt()], outs=[a2a_out.opt()],
                replica_groups=[list(range(CORES))])

        # ------------------------------- phase 3: output projection
        with ExitStack() as ph3:
            outp = ph3.enter_context(tc.tile_pool(name="outp", bufs=4))
            psP = ph3.enter_context(tc.tile_pool(name="psP", bufs=1, space="PSUM"))

            asrc = a2a_out.rearrange("g p q -> p g q")
            for q8 in range(8):
                sl = slice(q8 * HG // 8, (q8 + 1) * HG // 8)
                nc.sync.dma_start(out=attn_all[:, sl, :], in_=asrc[:, sl, :])

            for rt in range(NRT):
                pp = [psP.tile([128, OC], f32, tag=f"ppo{oc}", name=f"ppo{oc}")
                      for oc in range(NOC)]
                for g in range(HG):
                    al = attn_all[:, g, rt * 128:(rt + 1) * 128]
                    for oc in range(NOC):
                        nc.tensor.matmul(
                            pp[oc], lhsT=al, rhs=wo_all[:, g, oc * OC:(oc + 1) * OC],
                            start=(g == 0), stop=(g == HG - 1))
                for oc in range(NOC):
                    osb = outp.tile([128, OC], f32, tag="osb", name="osb")
                    nc.vector.tensor_copy(osb, pp[oc])
                    nc.sync.dma_start(out=out[rt * 128:(rt + 1) * 128, oc * OC:(oc + 1) * OC],
                                      in_=osb)

    nc.compile()
    return nc


def make_in_maps(x, cos, sin, Wq, Wk, Wv, Wo, QC):
    import ml_dtypes
    bf = ml_dtypes.bfloat16
    B, S, D = x.shape
    HD = cos.shape[1]
    H = Wq.shape[1] // HD
    QH = H // CORES
    NT = QC // 128
    R = B * S

    xT = np.ascontiguousarray(x.reshape(R, D).T).astype(bf)
    cosT = np.ascontiguousarray(cos.T).astype(bf)
    sT = sin.T.astype(np.float32)
    half = HD // 2
    sinTs = np.ascontiguousarray(
        np.concatenate([-sT[:half], sT[half:]], axis=0)).astype(bf)

    mk = np.zeros((128, NT * QC), dtype=np.float32)
    kk = np.arange(128)[:, None]
    qq = np.arange(QC)[None, :]
    for j in range(NT):
        mk[:, j * QC:(j + 1) * QC] = np.where(qq >= j * 128 + kk, 0.0, -1e9)
    mk = mk.astype(bf)
    ident = np.eye(128, dtype=np.float32).astype(bf)

    in_maps = []
    for c in range(CORES):
        in_maps.append({
            "xT": xT,
            "cosT": cosT,
            "sinT": sinTs,
            "wq": np.ascontiguousarray(Wq[:, c * QH * HD:(c + 1) * QH * HD]).astype(bf),
            "wk": np.ascontiguousarray(Wk[:, c * HD:(c + 1) * HD]).astype(bf),
            "wv": np.ascontiguousarray(Wv[:, c * HD:(c + 1) * HD]).astype(bf),
            "wo": np.asarray(Wo).astype(bf),
            "masks": mk,
            "ident": ident,
            "ones": np.ones((128, 128), dtype=bf),
        })
    return in_maps


def _install_profile_shim():
    """Provide antenv.axon_hooks (missing in this image) so
    run_bass_kernel_spmd(trace=True) can capture NTFF profiles via the
    axon PJRT .so; also neuter the artifact upload."""
    import types

    try:
        import antenv.axon_hooks  # noqa: F401
    except ImportError:
        from trn_agent_boot.trn_boot import _ntff_profile_via_ctypes
        hook = _ntff_profile_via_ctypes("/opt/axon/libaxon_pjrt.so")
        if hook is None:
            raise RuntimeError("libaxon_pjrt.so lacks profile symbols")
        mod = types.ModuleType("antenv.axon_hooks")
        mod.get_axon_ntff_profile_hook = lambda: hook
        mod.set_axon_ntff_profile_hook = lambda h: None
        sys.modules["antenv.axon_hooks"] = mod
        import antenv
        antenv.axon_hooks = mod
    import concourse.bass_utils as bu
    bu.upload_artifacts = lambda tmpdir: str(tmpdir)


_NC_CACHE = {}


def _get_nc(B, S, D, H, KV, HD, HO, QC):
    key = (B, S, D, H, KV, HD, HO, QC)
    if key not in _NC_CACHE:
        _NC_CACHE[key] = build_nc(B, S, D, H, KV, HD, HO, QC)
    return _NC_CACHE[key]


def kernel(x, cos, sin, Wq, Wk, Wv, Wo, _sim=False):
    x = np.asarray(x, dtype=np.float32)
    cos = np.asarray(cos, dtype=np.float32)
    sin = np.asarray(sin, dtype=np.float32)
    Wq = np.asarray(Wq, dtype=np.float32)
    Wk = np.asarray(Wk, dtype=np.float32)
    Wv = np.asarray(Wv, dtype=np.float32)
    Wo = np.asarray(Wo, dtype=np.float32)

    B, S, D = x.shape
    HD = cos.shape[1]
    H = Wq.shape[1] // HD
    KV = Wk.shape[1] // HD
    HO = Wq.shape[1]
    R = B * S
    QC = R // CORES

    nc = _get_nc(B, S, D, H, KV, HD, HO, QC)
    in_maps = make_in_maps(x, cos, sin, Wq, Wk, Wv, Wo, QC)

    if _sim:
        from concourse import bass_interp
        sim = bass_interp.MultiCoreSim(nc, CORES)
        for c in range(CORES):
            for k, v in in_maps[c].items():
                sim.cores[c].tensor(k)[:] = v
        sim.simulate(check_with_hw=False)
        shards = [np.array(sim.cores[c].mem_tensor("out")) for c in range(CORES)]
    else:
        from concourse.bass_utils import run_bass_kernel_spmd
        trace = os.environ.get("KERNEL_TRACE", "1") == "1"
        res = None
        if trace:
            try:
                _install_profile_shim()
                tmpdir = os.environ.get("KERNEL_TMPDIR") or None
                res = run_bass_kernel_spmd(nc, in_maps,
                                           core_ids=list(range(CORES)),
                                           trace=True, tmpdir=tmpdir)
            except Exception as e:  # fall back to untraced run
                print(f"traced run failed ({type(e).__name__}: {e}); "
                      f"retrying untraced")
                res = None
        if res is None:
            res = run_bass_kernel_spmd(nc, in_maps,
                                       core_ids=list(range(CORES)),
                                       trace=False)
        if res.exec_time_ns is not None:
            print(f"HW exec time: {res.exec_time_ns} ns")
        shards = [res.results[c]["out"] for c in range(CORES)]

    return np.concatenate(shards, axis=0).reshape(B, S, D).astype(np.float32)


# revision 23
# speedup vs baseline: 1.0377x; 1.0377x over previous
"""GQA causal attention (RoPE, B=2 S=2048 D=2048 H=16 KV=8 HD=128) on 8 trn2 cores.

Strategy: head-parallel. Each core c owns q-heads {2c, 2c+1} and kv-head c.
Host replicates x (pre-transposed to [D, B*S], bf16) to all cores; all
projections, RoPE and causal attention are head-sharded (zero comm). Two
AllToAlls (one per local q-head, 1 MB/rank each, bf16) convert the attention
output from head-sharding to row-sharding overlapped with the other head's
attention, then each core computes its 512-row slice of the output projection
with the full Wo. Host concatenates the 8 row shards.

Layout trick: everything is computed transposed (qT/kT = [HD, seq] with HD on
partitions, scores as [k, q]) so no on-device activation transposes are
needed; the only transposes are 128x128 PE transposes of vT -> v. Softmax runs
max-free (scores are small by construction), the denominator comes from a
ones-vector matmul on the PE, and the causal mask is added in PSUM via an
identity-matmul of a host-provided mask tile. All matmuls run bf16 (1 cyc/row
on the PE; fp32 accumulates in PSUM).
"""

import os
import sys

import numpy as np

if "/opt/trn_rl_repo" not in sys.path:
    sys.path.insert(0, "/opt/trn_rl_repo")

CORES = 8


def build_nc(B, S, D, H, KV, HD, HO, QC):
    """Build the SPMD bass graph (same graph for all 8 cores)."""
    import concourse.bacc as bacc
    import concourse.tile as tile
    from concourse import mybir
    from contextlib import ExitStack

    f32 = mybir.dt.float32
    bf16 = mybir.dt.bfloat16
    ACT = mybir.ActivationFunctionType

    QH = H // CORES               # q heads per core (2)
    R = B * S                     # total rows (4096)
    RO = R // CORES               # output rows per core (512) == QC
    assert QC == RO
    DK = D // 128                 # k-tiles over model dim (16)
    RC = 512                      # row-chunk width for projections
    NCH = R // RC                 # projection row chunks (8)
    NQC = S // QC                 # q chunks per batch (4)
    NKT = S // 128                # k tiles per batch (16)
    NT = QC // 128                # diagonal mask patterns (4)
    NRT = RO // 128               # out row tiles per core (4)
    OC = min(D, 512)              # out col chunk
    NOC = D // OC                 # out col chunks (4)
    HG = H                        # total heads in O-proj
    scale = float(HD) ** -0.5

    nc = bacc.Bacc("TRN2", target_bir_lowering=False, debug=False,
                   num_devices=CORES)

    xT = nc.dram_tensor("xT", [D, R], bf16, kind="ExternalInput")
    cosT = nc.dram_tensor("cosT", [HD, S], bf16, kind="ExternalInput")
    sinT = nc.dram_tensor("sinT", [HD, S], bf16, kind="ExternalInput")
    wq = nc.dram_tensor("wq", [D, QH * HD], bf16, kind="ExternalInput")
    wk = nc.dram_tensor("wk", [D, HD], bf16, kind="ExternalInput")
    wv = nc.dram_tensor("wv", [D, HD], bf16, kind="ExternalInput")
    wo = nc.dram_tensor("wo", [HO, D], bf16, kind="ExternalInput")
    masks = nc.dram_tensor("masks", [128, NT * QC], bf16, kind="ExternalInput")
    ident = nc.dram_tensor("ident", [128, 128], bf16, kind="ExternalInput")
    ones = nc.dram_tensor("ones", [128, 128], bf16, kind="ExternalInput")
    out = nc.dram_tensor("out", [RO, D], f32, kind="ExternalOutput")

    with tile.TileContext(nc) as tc, ExitStack() as top:
        dram = top.enter_context(tc.tile_pool(name="dram", bufs=1, space="DRAM"))
        consts = top.enter_context(tc.tile_pool(name="consts", bufs=1))
        resid = top.enter_context(tc.tile_pool(name="resid", bufs=1))

        a2a_in = dram.tile([CORES * QH, 128, QC], bf16)
        a2a_out = dram.tile([CORES * QH, 128, QC], bf16)

        ident_sb = consts.tile([128, 128], bf16)
        ones_sb = consts.tile([128, 128], bf16)
        mask_sb = consts.tile([128, NT * QC], bf16)

        # residents produced by projection phase, consumed by attention
        qT_sb = resid.tile([128, QH, R], bf16)       # [hd, head, row]
        kT_sb = resid.tile([128, R], bf16)           # [hd, row]
        v_sb = resid.tile([128, R], bf16)            # [kpos%128, ktile*HD+hd]

        # full Wo resident (prefetch recorded mid-projection; no deps)
        wo_all = resid.tile([128, HG, D], bf16)
        attn_all = resid.tile([128, HG, QC], bf16)

        # ------------------------------- phase 1: projections + rope
        with ExitStack() as ph1:
            ropec = ph1.enter_context(tc.tile_pool(name="ropec", bufs=1))
            wpool = ph1.enter_context(tc.tile_pool(name="wpool", bufs=1))
            xpool = ph1.enter_context(tc.tile_pool(name="xpool", bufs=5))
            rtmp = ph1.enter_context(tc.tile_pool(name="rtmp", bufs=2))
            psA = ph1.enter_context(tc.tile_pool(name="psA", bufs=4, space="PSUM"))
            psTR = ph1.enter_context(tc.tile_pool(name="psTR", bufs=2, space="PSUM"))

            wq_sb = wpool.tile([128, DK, QH * HD], bf16)
            wk_sb = wpool.tile([128, DK, HD], bf16)
            wv_sb = wpool.tile([128, DK, HD], bf16)
            cos_sb = ropec.tile([128, S], bf16)
            sin_sb = ropec.tile([128, S], bf16)
            wq_r = wq.ap().rearrange("(kt p) c -> p kt c", p=128)
            wk_r = wk.ap().rearrange("(kt p) c -> p kt c", p=128)
            wv_r = wv.ap().rearrange("(kt p) c -> p kt c", p=128)

            # chunk-0 x + weights interleaved kt-granular so MM kt can start
            # as soon as its own pieces land (subtile deps)
            DKH = DK // 2
            xch0 = [xpool.tile([128, DKH, RC], bf16, tag="xch", name="xch0")
                    for _ in range(2)]
            xsrc0 = xT[:, 0:RC].rearrange("(kt p) c -> p kt c", p=128)
            ncs = S // 512
            for kt in range(DK):
                nc.sync.dma_start(out=xch0[kt // DKH][:, kt % DKH, :],
                                  in_=xsrc0[:, kt, :])
                nc.sync.dma_start(out=wq_sb[:, kt, :], in_=wq_r[:, kt, :])
                nc.sync.dma_start(out=wk_sb[:, kt, :], in_=wk_r[:, kt, :])
                nc.sync.dma_start(out=wv_sb[:, kt, :], in_=wv_r[:, kt, :])
                if kt % 4 == 3:
                    cs = kt // 4
                    if cs < ncs:
                        sl = slice(cs * 512, (cs + 1) * 512)
                        nc.sync.dma_start(out=cos_sb[:, sl], in_=cosT[:, sl])
                        nc.sync.dma_start(out=sin_sb[:, sl], in_=sinT[:, sl])
            for cs in range(DK // 4, ncs):
                sl = slice(cs * 512, (cs + 1) * 512)
                nc.sync.dma_start(out=cos_sb[:, sl], in_=cosT[:, sl])
                nc.sync.dma_start(out=sin_sb[:, sl], in_=sinT[:, sl])
            nc.sync.dma_start(out=ident_sb, in_=ident[:, :])
            nc.sync.dma_start(out=ones_sb, in_=ones[:, :])
            nc.sync.dma_start(out=mask_sb, in_=masks[:, :])

            half = HD // 2

            def rope(pp, dst, poff):
                c_sl = cos_sb[:, poff:poff + RC]
                s_sl = sin_sb[:, poff:poff + RC]
                t1 = rtmp.tile([128, RC], f32, tag="t1", name="t1")
                t2 = rtmp.tile([128, RC], f32, tag="t2", name="t2")
                nc.vector.tensor_mul(t1, pp, c_sl)
                nc.vector.tensor_mul(t2[0:half, :], pp[half:128, :], s_sl[0:half, :])
                nc.vector.tensor_mul(t2[half:128, :], pp[0:half, :], s_sl[half:128, :])
                nc.vector.tensor_add(dst, t1, t2)

            for n in range(NCH):
                if n == 0:
                    xs = xch0
                else:
                    xsrc = xT[:, n * RC:(n + 1) * RC].rearrange(
                        "(kt p) c -> p kt c", p=128)
                    xs = []
                    for hh in range(2):
                        xc = xpool.tile([128, DKH, RC], bf16, tag="xch", name="xch")
                        for q4 in range(4):
                            sl = slice(q4 * DKH // 4, (q4 + 1) * DKH // 4)
                            nc.sync.dma_start(
                                out=xc[:, sl, :],
                                in_=xsrc[:, hh * DKH + q4 * DKH // 4:
                                         hh * DKH + (q4 + 1) * DKH // 4, :])
                        xs.append(xc)
                if n == min(2, NCH - 1):
                    # prefetch Wo now: pipeline warm, sync queues mostly free
                    wo_r = wo.ap().rearrange("(g p) n -> p g n", p=128)
                    for q8 in range(8):
                        sl = slice(q8 * HG // 8, (q8 + 1) * HG // 8)
                        nc.sync.dma_start(out=wo_all[:, sl, :], in_=wo_r[:, sl, :])

                poff = (n * RC) % S
                for oi in range(QH + 2):   # QH q heads, then k, then vT
                    pp = psA.tile([128, RC], f32, tag="pp", name="pp")
                    if oi < QH:
                        wcol = (wq_sb, oi * HD)
                    elif oi == QH:
                        wcol = (wk_sb, 0)
                    else:
                        wcol = (wv_sb, 0)
                    for kt in range(DK):
                        wsb = wcol[0][:, kt, wcol[1]:wcol[1] + HD]
                        nc.tensor.matmul(
                            pp, lhsT=wsb, rhs=xs[kt // DKH][:, kt % DKH, :],
                            start=(kt == 0), stop=(kt == DK - 1))
                    if oi < QH:
                        rope(pp, qT_sb[:, oi, n * RC:(n + 1) * RC], poff)
                    elif oi == QH:
                        rope(pp, kT_sb[:, n * RC:(n + 1) * RC], poff)
                    else:
                        vt_sb = rtmp.tile([128, RC], bf16, tag="vt", name="vt")
                        nc.scalar.activation(vt_sb, pp, ACT.Copy)
                        for j in range(RC // 128):
                            ptr_ = psTR.tile([128, 128], bf16, tag="ptr", name="ptr")
                            nc.tensor.transpose(ptr_, vt_sb[:, j * 128:(j + 1) * 128], ident_sb)
                            rti = n * (RC // 128) + j
                            nc.scalar.activation(v_sb[:, rti * 128:(rti + 1) * 128], ptr_, ACT.Copy)

        # ------------------------------- phase 2: attention (h-paired)
        with ExitStack() as ph2:
            probs = ph2.enter_context(tc.tile_pool(name="probs", bufs=36))
            atmp = ph2.enter_context(tc.tile_pool(name="atmp", bufs=3))
            dens = ph2.enter_context(tc.tile_pool(name="dens", bufs=2))
            psS = ph2.enter_context(tc.tile_pool(name="psS", bufs=3, space="PSUM"))
            psO = ph2.enter_context(tc.tile_pool(name="psO", bufs=1, space="PSUM"))
            psD = ph2.enter_context(tc.tile_pool(name="psD", bufs=1, space="PSUM"))
            psB = ph2.enter_context(tc.tile_pool(name="psB", bufs=1, space="PSUM"))

            from concourse import mybir as _mb
            for b in range(B):
                for qc in range(NQC - 1, -1, -1):
                    nkt = (qc + 1) * NT
                    po = [psO.tile([128, QC], f32, tag=f"po{h}", name=f"po{h}")
                          for h in range(QH)]
                    pden = [psD.tile([1, QC], f32, tag=f"pden{h}", name=f"pden{h}")
                            for h in range(QH)]
                    prs = {}
                    offs = {}
                    # scores + exp (kT ldweights shared across heads)
                    for kt in range(nkt):
                        dj = kt - qc * NT   # >=0 on diagonal block
                        o = max(dj, 0) * 128   # first valid q col in chunk
                        kl = kT_sb[:, b * S + kt * 128: b * S + (kt + 1) * 128]
                        for h in range(QH):
                            sc = psS.tile([128, QC], f32, tag="sc", name="sc")
                            nc.tensor.matmul(
                                sc[:, o:QC], lhsT=kl,
                                rhs=qT_sb[:, h, b * S + qc * QC + o: b * S + (qc + 1) * QC],
                                start=True, stop=(dj < 0))
                            if dj >= 0:
                                nc.tensor.matmul(
                                    sc[:, o:QC], lhsT=ident_sb,
                                    rhs=mask_sb[:, dj * QC + o:(dj + 1) * QC],
                                    start=False, stop=True)
                            pr = probs.tile([128, QC], bf16, tag="pr", name="pr")
                            nc.scalar.activation(pr[:, o:QC], sc[:, o:QC],
                                                 ACT.Exp, scale=scale)
                            prs[(h, kt)] = pr
                            offs[kt] = o
                    # PV accumulation (v ldweights shared across heads)
                    for kt in range(nkt):
                        ktg = b * NKT + kt
                        o = offs[kt]
                        vl = v_sb[:, ktg * 128:(ktg + 1) * 128]
                        for h in range(QH):
                            nc.tensor.matmul(
                                po[h][:, o:QC], lhsT=vl, rhs=prs[(h, kt)][:, o:QC],
                                start=(kt == 0), stop=(kt == nkt - 1))
                    # denominators (ones ldweights shared across whole batch)
                    for h in range(QH):
                        for kt in range(nkt):
                            o = offs[kt]
                            nc.tensor.matmul(
                                pden[h][:, o:QC], lhsT=ones_sb[:, 0:1],
                                rhs=prs[(h, kt)][:, o:QC],
                                start=(kt == 0), stop=(kt == nkt - 1))
                    # normalize + ship to A2A bounce.  Copy po out of PSUM
                    # first (frees the accumulator bank for the next chunk
                    # without waiting on the reciprocal chain).
                    d = b * NQC + qc   # dest core for these q rows
                    for h in range(QH):
                        at = atmp.tile([128, QC], bf16, tag="at", name="at")
                        nc.scalar.activation(at, po[h], ACT.Copy)
                        den = dens.tile([1, QC], f32, tag="den", name="den")
                        nc.vector.reciprocal_approx_fast(den, pden[h])
                        den_b = dens.tile([1, QC], bf16, tag="denb", name="den_b")
                        nc.scalar.activation(den_b, den, ACT.Copy)
                        pbc = psB.tile([128, QC], f32, tag="pbc", name="pbc")
                        nc.tensor.matmul(pbc, lhsT=ones_sb[0:1, :], rhs=den_b,
                                         start=True, stop=True)
                        bc = atmp.tile([128, QC], bf16, tag="bc", name="bc")
                        nc.scalar.activation(bc, pbc, ACT.Copy)
                        anorm = atmp.tile([128, QC], bf16, tag="an", name="anorm")
                        nc.vector.tensor_mul(anorm, at, bc)
                        nc.sync.dma_start(out=a2a_in[d * QH + h], in_=anorm)

            nc.gpsimd.collective_compute(
                "AllToAll", _mb.AluOpType.bypass,
                ins=[a2a_in.opt()], outs=[a2a_out.opt()],
                replica_groups=[list(range(CORES))])

        # ------------------------------- phase 3: output projection
        with ExitStack() as ph3:
            outp = ph3.enter_context(tc.tile_pool(name="outp", bufs=4))
            psP = ph3.enter_context(tc.tile_pool(name="psP", bufs=1, space="PSUM"))

            asrc = a2a_out.rearrange("g p q -> p g q")
            for q8 in range(8):
                sl = slice(q8 * HG // 8, (q8 + 1) * HG // 8)
                nc.sync.dma_start(out=attn_all[:, sl, :], in_=asrc[:, sl, :])

            for rt in range(NRT):
                pp = [psP.tile([128, OC], f32, tag=f"ppo{oc}", name=f"ppo{oc}")
                      for oc in range(NOC)]
                for g in range(HG):
                    al = attn_all[:, g, rt * 128:(rt + 1) * 128]
                    for oc in range(NOC):
                        nc.tensor.matmul(
                            pp[oc], lhsT=al, rhs=wo_all[:, g, oc * OC:(oc + 1) * OC],
                            start=(g == 0), stop=(g == HG - 1))
                for oc in range(NOC):
                    osb = outp.tile([128, OC], f32, tag="osb", name="osb")
                    nc.vector.tensor_copy(osb, pp[oc])
                    nc.sync.dma_start(out=out[rt * 128:(rt + 1) * 128, oc * OC:(oc + 1) * OC],
                                      in_=osb)

    nc.compile()
    return nc


def make_in_maps(x, cos, sin, Wq, Wk, Wv, Wo, QC):
    import ml_dtypes
    bf = ml_dtypes.bfloat16
    B, S, D = x.shape
    HD = cos.shape[1]
    H = Wq.shape[1] // HD
    QH = H // CORES
    NT = QC // 128
    R = B * S

    xT = np.ascontiguousarray(x.reshape(R, D).T).astype(bf)
    cosT = np.ascontiguousarray(cos.T).astype(bf)
    sT = sin.T.astype(np.float32)
    half = HD // 2
    sinTs = np.ascontiguousarray(
        np.concatenate([-sT[:half], sT[half:]], axis=0)).astype(bf)

    mk = np.zeros((128, NT * QC), dtype=np.float32)
    kk = np.arange(128)[:, None]
    qq = np.arange(QC)[None, :]
    for j in range(NT):
        mk[:, j * QC:(j + 1) * QC] = np.where(qq >= j * 128 + kk, 0.0, -1e9)
    mk = mk.astype(bf)
    ident = np.eye(128, dtype=np.float32).astype(bf)

    in_maps = []
    for c in range(CORES):
        in_maps.append({
            "xT": xT,
            "cosT": cosT,
            "sinT": sinTs,
            "wq": np.ascontiguousarray(Wq[:, c * QH * HD:(c + 1) * QH * HD]).astype(bf),
            "wk": np.ascontiguousarray(Wk[:, c * HD:(c + 1) * HD]).astype(bf),
            "wv": np.ascontiguousarray(Wv[:, c * HD:(c + 1) * HD]).astype(bf),
            "wo": np.asarray(Wo).astype(bf),
            "masks": mk,
            "ident": ident,
            "ones": np.ones((128, 128), dtype=bf),
        })
    return in_maps


def _install_profile_shim():
    """Provide antenv.axon_hooks (missing in this image) so
    run_bass_kernel_spmd(trace=True) can capture NTFF profiles via the
    axon PJRT .so; also neuter the artifact upload."""
    import types

    try:
        import antenv.axon_hooks  # noqa: F401
    except ImportError:
        from trn_agent_boot.trn_boot import _ntff_profile_via_ctypes
        hook = _ntff_profile_via_ctypes("/opt/axon/libaxon_pjrt.so")
        if hook is None:
            raise RuntimeError("libaxon_pjrt.so lacks profile symbols")
        mod = types.ModuleType("antenv.axon_hooks")
        mod.get_axon_ntff_profile_hook = lambda: hook
        mod.set_axon_ntff_profile_hook = lambda h: None
        sys.modules["antenv.axon_hooks"] = mod
        import antenv
        antenv.axon_hooks = mod
    import concourse.bass_utils as bu
    bu.upload_artifacts = lambda tmpdir: str(tmpdir)


_NC_CACHE = {}


def _get_nc(B, S, D, H, KV, HD, HO, QC):
    key = (B, S, D, H, KV, HD, HO, QC)
    if key not in _NC_CACHE:
        _NC_CACHE[key] = build_nc(B, S, D, H, KV, HD, HO, QC)
    return _NC_CACHE[key]


def kernel(x, cos, sin, Wq, Wk, Wv, Wo, _sim=False):
    x = np.asarray(x, dtype=np.float32)
    cos = np.asarray(cos, dtype=np.float32)
    sin = np.asarray(sin, dtype=np.float32)
    Wq = np.asarray(Wq, dtype=np.float32)
    Wk = np.asarray(Wk, dtype=np.float32)
    Wv = np.asarray(Wv, dtype=np.float32)
    Wo = np.asarray(Wo, dtype=np.float32)

    B, S, D = x.shape
    HD = cos.shape[1]
    H = Wq.shape[1] // HD
    KV = Wk.shape[1] // HD
    HO = Wq.shape[1]
    R = B * S
    QC = R // CORES

    nc = _get_nc(B, S, D, H, KV, HD, HO, QC)
    in_maps = make_in_maps(x, cos, sin, Wq, Wk, Wv, Wo, QC)

    if _sim:
        from concourse import bass_interp
        sim = bass_interp.MultiCoreSim(nc, CORES)
        for c in range(CORES):
            for k, v in in_maps[c].items():
                sim.cores[c].tensor(k)[:] = v
        sim.simulate(check_with_hw=False)
        shards = [np.array(sim.cores[c].mem_tensor("out")) for c in range(CORES)]
    else:
        from concourse.bass_utils import run_bass_kernel_spmd
        trace = os.environ.get("KERNEL_TRACE", "0") == "1"
        res = None
        if trace:
            try:
                _install_profile_shim()
                tmpdir = os.environ.get("KERNEL_TMPDIR") or None
                res = run_bass_kernel_spmd(nc, in_maps,
                                           core_ids=list(range(CORES)),
                                           trace=True, tmpdir=tmpdir)
            except Exception as e:  # fall back to untraced run
                print(f"traced run failed ({type(e).__name__}: {e}); "
                      f"retrying untraced")
                res = None
        if res is None:
            res = run_bass_kernel_spmd(nc, in_maps,
                                       core_ids=list(range(CORES)),
                                       trace=False)
        if res.exec_time_ns is not None:
            print(f"HW exec time: {res.exec_time_ns} ns")
        shards = [res.results[c]["out"] for c in range(CORES)]

    return np.concatenate(shards, axis=0).reshape(B, S, D).astype(np.float32)


# revision 24
# speedup vs baseline: 1.0962x; 1.0564x over previous
"""GQA causal attention (RoPE, B=2 S=2048 D=2048 H=16 KV=8 HD=128) on 8 trn2 cores.

Strategy: head-parallel. Each core c owns q-heads {2c, 2c+1} and kv-head c.
Host replicates x (pre-transposed to [D, B*S], bf16) to all cores; all
projections, RoPE and causal attention are head-sharded (zero comm). Two
AllToAlls (one per local q-head, 1 MB/rank each, bf16) convert the attention
output from head-sharding to row-sharding overlapped with the other head's
attention, then each core computes its 512-row slice of the output projection
with the full Wo. Host concatenates the 8 row shards.

Layout trick: everything is computed transposed (qT/kT = [HD, seq] with HD on
partitions, scores as [k, q]) so no on-device activation transposes are
needed; the only transposes are 128x128 PE transposes of vT -> v. Softmax runs
max-free (scores are small by construction), the denominator comes from a
ones-vector matmul on the PE, and the causal mask is added in PSUM via an
identity-matmul of a host-provided mask tile. All matmuls run bf16 (1 cyc/row
on the PE; fp32 accumulates in PSUM).
"""

import os
import sys

import numpy as np

if "/opt/trn_rl_repo" not in sys.path:
    sys.path.insert(0, "/opt/trn_rl_repo")

CORES = 8


def build_nc(B, S, D, H, KV, HD, HO, QC):
    """Build the SPMD bass graph (same graph for all 8 cores)."""
    import concourse.bacc as bacc
    import concourse.tile as tile
    from concourse import mybir
    from contextlib import ExitStack

    f32 = mybir.dt.float32
    bf16 = mybir.dt.bfloat16
    ACT = mybir.ActivationFunctionType

    QH = H // CORES               # q heads per core (2)
    R = B * S                     # total rows (4096)
    RO = R // CORES               # output rows per core (512) == QC
    assert QC == RO
    DK = D // 128                 # k-tiles over model dim (16)
    RC = 512                      # row-chunk width for projections
    NCH = R // RC                 # projection row chunks (8)
    NQC = S // QC                 # q chunks per batch (4)
    NKT = S // 128                # k tiles per batch (16)
    NT = QC // 128                # diagonal mask patterns (4)
    NRT = RO // 128               # out row tiles per core (4)
    OC = min(D, 512)              # out col chunk
    NOC = D // OC                 # out col chunks (4)
    HG = H                        # total heads in O-proj
    scale = float(HD) ** -0.5

    nc = bacc.Bacc("TRN2", target_bir_lowering=False, debug=False,
                   num_devices=CORES)

    xT = nc.dram_tensor("xT", [D, R], bf16, kind="ExternalInput")
    cosT = nc.dram_tensor("cosT", [HD, S], bf16, kind="ExternalInput")
    sinT = nc.dram_tensor("sinT", [HD, S], bf16, kind="ExternalInput")
    wq = nc.dram_tensor("wq", [D, QH * HD], bf16, kind="ExternalInput")
    wk = nc.dram_tensor("wk", [D, HD], bf16, kind="ExternalInput")
    wv = nc.dram_tensor("wv", [D, HD], bf16, kind="ExternalInput")
    wo = nc.dram_tensor("wo", [HO, D], bf16, kind="ExternalInput")
    masks = nc.dram_tensor("masks", [128, NT * QC], bf16, kind="ExternalInput")
    ident = nc.dram_tensor("ident", [128, 128], bf16, kind="ExternalInput")
    ones = nc.dram_tensor("ones", [128, 128], bf16, kind="ExternalInput")
    out = nc.dram_tensor("out", [RO, D], f32, kind="ExternalOutput")

    with tile.TileContext(nc) as tc, ExitStack() as top:
        dram = top.enter_context(tc.tile_pool(name="dram", bufs=1, space="DRAM"))
        consts = top.enter_context(tc.tile_pool(name="consts", bufs=1))
        resid = top.enter_context(tc.tile_pool(name="resid", bufs=1))

        a2a_in = [dram.tile([CORES, 128, QC], bf16, name=f"a2ain{h}")
                  for h in range(QH)]
        a2a_out = [dram.tile([CORES, 128, QC], bf16, name=f"a2aout{h}")
                   for h in range(QH)]

        ident_sb = consts.tile([128, 128], bf16)
        ones_sb = consts.tile([128, 128], bf16)
        mask_sb = consts.tile([128, NT * QC], bf16)

        # residents produced by projection phase, consumed by attention
        qT_sb = resid.tile([128, QH, R], bf16)       # [hd, head, row]
        kT_sb = resid.tile([128, R], bf16)           # [hd, row]
        v_sb = resid.tile([128, R], bf16)            # [kpos%128, ktile*HD+hd]

        # full Wo resident (prefetch recorded mid-projection; no deps)
        wo_all = resid.tile([128, HG, D], bf16)
        attn_all = resid.tile([128, HG, QC], bf16)

        # ------------------------------- phase 1: projections + rope
        with ExitStack() as ph1:
            ropec = ph1.enter_context(tc.tile_pool(name="ropec", bufs=1))
            wpool = ph1.enter_context(tc.tile_pool(name="wpool", bufs=1))
            xpool = ph1.enter_context(tc.tile_pool(name="xpool", bufs=5))
            rtmp = ph1.enter_context(tc.tile_pool(name="rtmp", bufs=2))
            psA = ph1.enter_context(tc.tile_pool(name="psA", bufs=4, space="PSUM"))
            psTR = ph1.enter_context(tc.tile_pool(name="psTR", bufs=2, space="PSUM"))

            wq_sb = wpool.tile([128, DK, QH * HD], bf16)
            wk_sb = wpool.tile([128, DK, HD], bf16)
            wv_sb = wpool.tile([128, DK, HD], bf16)
            cos_sb = ropec.tile([128, S], bf16)
            sin_sb = ropec.tile([128, S], bf16)
            wq_r = wq.ap().rearrange("(kt p) c -> p kt c", p=128)
            wk_r = wk.ap().rearrange("(kt p) c -> p kt c", p=128)
            wv_r = wv.ap().rearrange("(kt p) c -> p kt c", p=128)

            # chunk-0 x + weights interleaved kt-granular so MM kt can start
            # as soon as its own pieces land (subtile deps)
            DKH = DK // 2
            xch0 = [xpool.tile([128, DKH, RC], bf16, tag="xch", name="xch0")
                    for _ in range(2)]
            xsrc0 = xT[:, 0:RC].rearrange("(kt p) c -> p kt c", p=128)
            ncs = S // 512
            for kt in range(DK):
                nc.sync.dma_start(out=xch0[kt // DKH][:, kt % DKH, :],
                                  in_=xsrc0[:, kt, :])
                nc.sync.dma_start(out=wq_sb[:, kt, :], in_=wq_r[:, kt, :])
                nc.sync.dma_start(out=wk_sb[:, kt, :], in_=wk_r[:, kt, :])
                nc.sync.dma_start(out=wv_sb[:, kt, :], in_=wv_r[:, kt, :])
                if kt % 4 == 3:
                    cs = kt // 4
                    if cs < ncs:
                        sl = slice(cs * 512, (cs + 1) * 512)
                        nc.sync.dma_start(out=cos_sb[:, sl], in_=cosT[:, sl])
                        nc.sync.dma_start(out=sin_sb[:, sl], in_=sinT[:, sl])
            for cs in range(DK // 4, ncs):
                sl = slice(cs * 512, (cs + 1) * 512)
                nc.sync.dma_start(out=cos_sb[:, sl], in_=cosT[:, sl])
                nc.sync.dma_start(out=sin_sb[:, sl], in_=sinT[:, sl])
            nc.sync.dma_start(out=ident_sb, in_=ident[:, :])
            nc.sync.dma_start(out=ones_sb, in_=ones[:, :])
            nc.sync.dma_start(out=mask_sb, in_=masks[:, :])

            half = HD // 2

            def rope(pp, dst, poff):
                c_sl = cos_sb[:, poff:poff + RC]
                s_sl = sin_sb[:, poff:poff + RC]
                t1 = rtmp.tile([128, RC], f32, tag="t1", name="t1")
                t2 = rtmp.tile([128, RC], f32, tag="t2", name="t2")
                nc.vector.tensor_mul(t1, pp, c_sl)
                nc.vector.tensor_mul(t2[0:half, :], pp[half:128, :], s_sl[0:half, :])
                nc.vector.tensor_mul(t2[half:128, :], pp[0:half, :], s_sl[half:128, :])
                nc.vector.tensor_add(dst, t1, t2)

            for n in range(NCH):
                if n == 0:
                    xs = xch0
                else:
                    xsrc = xT[:, n * RC:(n + 1) * RC].rearrange(
                        "(kt p) c -> p kt c", p=128)
                    xs = []
                    for hh in range(2):
                        xc = xpool.tile([128, DKH, RC], bf16, tag="xch", name="xch")
                        for q4 in range(4):
                            sl = slice(q4 * DKH // 4, (q4 + 1) * DKH // 4)
                            nc.sync.dma_start(
                                out=xc[:, sl, :],
                                in_=xsrc[:, hh * DKH + q4 * DKH // 4:
                                         hh * DKH + (q4 + 1) * DKH // 4, :])
                        xs.append(xc)
                if n == min(2, NCH - 1):
                    # prefetch Wo now: pipeline warm, sync queues mostly free
                    wo_r = wo.ap().rearrange("(g p) n -> p g n", p=128)
                    for q8 in range(8):
                        sl = slice(q8 * HG // 8, (q8 + 1) * HG // 8)
                        nc.sync.dma_start(out=wo_all[:, sl, :], in_=wo_r[:, sl, :])

                poff = (n * RC) % S
                for oi in range(QH + 2):   # QH q heads, then k, then vT
                    pp = psA.tile([128, RC], f32, tag="pp", name="pp")
                    if oi < QH:
                        wcol = (wq_sb, oi * HD)
                    elif oi == QH:
                        wcol = (wk_sb, 0)
                    else:
                        wcol = (wv_sb, 0)
                    for kt in range(DK):
                        wsb = wcol[0][:, kt, wcol[1]:wcol[1] + HD]
                        nc.tensor.matmul(
                            pp, lhsT=wsb, rhs=xs[kt // DKH][:, kt % DKH, :],
                            start=(kt == 0), stop=(kt == DK - 1))
                    if oi < QH:
                        rope(pp, qT_sb[:, oi, n * RC:(n + 1) * RC], poff)
                    elif oi == QH:
                        rope(pp, kT_sb[:, n * RC:(n + 1) * RC], poff)
                    else:
                        vt_sb = rtmp.tile([128, RC], bf16, tag="vt", name="vt")
                        nc.scalar.activation(vt_sb, pp, ACT.Copy)
                        for j in range(RC // 128):
                            ptr_ = psTR.tile([128, 128], bf16, tag="ptr", name="ptr")
                            nc.tensor.transpose(ptr_, vt_sb[:, j * 128:(j + 1) * 128], ident_sb)
                            rti = n * (RC // 128) + j
                            nc.scalar.activation(v_sb[:, rti * 128:(rti + 1) * 128], ptr_, ACT.Copy)

        # ------------------------------- phase 2: attention (h-outer; the
        # first head's AllToAll overlaps the second head's attention)
        with ExitStack() as ph2:
            probs = ph2.enter_context(tc.tile_pool(name="probs", bufs=20))
            atmp = ph2.enter_context(tc.tile_pool(name="atmp", bufs=3))
            dens = ph2.enter_context(tc.tile_pool(name="dens", bufs=2))
            psS = ph2.enter_context(tc.tile_pool(name="psS", bufs=3, space="PSUM"))
            psO = ph2.enter_context(tc.tile_pool(name="psO", bufs=2, space="PSUM"))
            psD = ph2.enter_context(tc.tile_pool(name="psD", bufs=2, space="PSUM"))
            psB = ph2.enter_context(tc.tile_pool(name="psB", bufs=1, space="PSUM"))

            from concourse import mybir as _mb
            for h in range(QH):
                for b in range(B):
                    for qc in range(NQC - 1, -1, -1):
                        nkt = (qc + 1) * NT
                        po = psO.tile([128, QC], f32, tag="po", name="po")
                        pden = psD.tile([1, QC], f32, tag="pden", name="pden")
                        prs = {}
                        offs = {}
                        for kt in range(nkt):
                            dj = kt - qc * NT   # >=0 on diagonal block
                            o = max(dj, 0) * 128
                            kl = kT_sb[:, b * S + kt * 128: b * S + (kt + 1) * 128]
                            sc = psS.tile([128, QC], f32, tag="sc", name="sc")
                            nc.tensor.matmul(
                                sc[:, o:QC], lhsT=kl,
                                rhs=qT_sb[:, h, b * S + qc * QC + o: b * S + (qc + 1) * QC],
                                start=True, stop=(dj < 0))
                            if dj >= 0:
                                nc.tensor.matmul(
                                    sc[:, o:QC], lhsT=ident_sb,
                                    rhs=mask_sb[:, dj * QC + o:(dj + 1) * QC],
                                    start=False, stop=True)
                            pr = probs.tile([128, QC], bf16, tag="pr", name="pr")
                            nc.scalar.activation(pr[:, o:QC], sc[:, o:QC],
                                                 ACT.Exp, scale=scale)
                            prs[kt] = pr
                            offs[kt] = o
                        for kt in range(nkt):
                            ktg = b * NKT + kt
                            o = offs[kt]
                            nc.tensor.matmul(
                                po[:, o:QC], lhsT=v_sb[:, ktg * 128:(ktg + 1) * 128],
                                rhs=prs[kt][:, o:QC],
                                start=(kt == 0), stop=(kt == nkt - 1))
                        for kt in range(nkt):
                            o = offs[kt]
                            nc.tensor.matmul(
                                pden[:, o:QC], lhsT=ones_sb[:, 0:1],
                                rhs=prs[kt][:, o:QC],
                                start=(kt == 0), stop=(kt == nkt - 1))
                        # normalize; drain po out of PSUM immediately
                        d = b * NQC + qc
                        at = atmp.tile([128, QC], bf16, tag="at", name="at")
                        nc.scalar.activation(at, po, ACT.Copy)
                        den = dens.tile([1, QC], f32, tag="den", name="den")
                        nc.vector.reciprocal_approx_fast(den, pden)
                        den_b = dens.tile([1, QC], bf16, tag="denb", name="den_b")
                        nc.scalar.activation(den_b, den, ACT.Copy)
                        pbc = psB.tile([128, QC], f32, tag="pbc", name="pbc")
                        nc.tensor.matmul(pbc, lhsT=ones_sb[0:1, :], rhs=den_b,
                                         start=True, stop=True)
                        bc = atmp.tile([128, QC], bf16, tag="bc", name="bc")
                        nc.scalar.activation(bc, pbc, ACT.Copy)
                        anorm = atmp.tile([128, QC], bf16, tag="an", name="anorm")
                        nc.vector.tensor_mul(anorm, at, bc)
                        nc.sync.dma_start(out=a2a_in[h][d], in_=anorm)

                nc.gpsimd.collective_compute(
                    "AllToAll", _mb.AluOpType.bypass,
                    ins=[a2a_in[h].opt()], outs=[a2a_out[h].opt()],
                    replica_groups=[list(range(CORES))])
                # land this head's share of attn_all right away
                asrc = a2a_out[h].rearrange("g p q -> p g q")
                adst = attn_all.rearrange("p (g hl) q -> p g hl q", hl=QH)
                for q4 in range(4):
                    sl = slice(q4 * CORES // 4, (q4 + 1) * CORES // 4)
                    nc.sync.dma_start(out=adst[:, sl, h, :], in_=asrc[:, sl, :])

        # ------------------------------- phase 3: output projection
        with ExitStack() as ph3:
            outp = ph3.enter_context(tc.tile_pool(name="outp", bufs=4))
            psP = ph3.enter_context(tc.tile_pool(name="psP", bufs=2, space="PSUM"))

            g_order = [g for g in range(HG) if g % QH == 0] + \
                      [g for g in range(HG) if g % QH != 0]
            for rt in range(NRT):
                pp = [psP.tile([128, OC], f32, tag=f"ppo{oc}", name=f"ppo{oc}")
                      for oc in range(NOC)]
                for gi, g in enumerate(g_order):
                    al = attn_all[:, g, rt * 128:(rt + 1) * 128]
                    for oc in range(NOC):
                        nc.tensor.matmul(
                            pp[oc], lhsT=al, rhs=wo_all[:, g, oc * OC:(oc + 1) * OC],
                            start=(gi == 0), stop=(gi == HG - 1))
                for oc in range(NOC):
                    osb = outp.tile([128, OC], f32, tag="osb", name="osb")
                    nc.vector.tensor_copy(osb, pp[oc])
                    nc.sync.dma_start(out=out[rt * 128:(rt + 1) * 128, oc * OC:(oc + 1) * OC],
                                      in_=osb)

    nc.compile()
    return nc


def make_in_maps(x, cos, sin, Wq, Wk, Wv, Wo, QC):
    import ml_dtypes
    bf = ml_dtypes.bfloat16
    B, S, D = x.shape
    HD = cos.shape[1]
    H = Wq.shape[1] // HD
    QH = H // CORES
    NT = QC // 128
    R = B * S

    xT = np.ascontiguousarray(x.reshape(R, D).T).astype(bf)
    cosT = np.ascontiguousarray(cos.T).astype(bf)
    sT = sin.T.astype(np.float32)
    half = HD // 2
    sinTs = np.ascontiguousarray(
        np.concatenate([-sT[:half], sT[half:]], axis=0)).astype(bf)

    mk = np.zeros((128, NT * QC), dtype=np.float32)
    kk = np.arange(128)[:, None]
    qq = np.arange(QC)[None, :]
    for j in range(NT):
        mk[:, j * QC:(j + 1) * QC] = np.where(qq >= j * 128 + kk, 0.0, -1e9)
    mk = mk.astype(bf)
    ident = np.eye(128, dtype=np.float32).astype(bf)

    in_maps = []
    for c in range(CORES):
        in_maps.append({
            "xT": xT,
            "cosT": cosT,
            "sinT": sinTs,
            "wq": np.ascontiguousarray(Wq[:, c * QH * HD:(c + 1) * QH * HD]).astype(bf),
            "wk": np.ascontiguousarray(Wk[:, c * HD:(c + 1) * HD]).astype(bf),
            "wv": np.ascontiguousarray(Wv[:, c * HD:(c + 1) * HD]).astype(bf),
            "wo": np.asarray(Wo).astype(bf),
            "masks": mk,
            "ident": ident,
            "ones": np.ones((128, 128), dtype=bf),
        })
    return in_maps


def _install_profile_shim():
    """Provide antenv.axon_hooks (missing in this image) so
    run_bass_kernel_spmd(trace=True) can capture NTFF profiles via the
    axon PJRT .so; also neuter the artifact upload."""
    import types

    try:
        import antenv.axon_hooks  # noqa: F401
    except ImportError:
        from trn_agent_boot.trn_boot import _ntff_profile_via_ctypes
        hook = _ntff_profile_via_ctypes("/opt/axon/libaxon_pjrt.so")
        if hook is None:
            raise RuntimeError("libaxon_pjrt.so lacks profile symbols")
        mod = types.ModuleType("antenv.axon_hooks")
        mod.get_axon_ntff_profile_hook = lambda: hook
        mod.set_axon_ntff_profile_hook = lambda h: None
        sys.modules["antenv.axon_hooks"] = mod
        import antenv
        antenv.axon_hooks = mod
    import concourse.bass_utils as bu
    bu.upload_artifacts = lambda tmpdir: str(tmpdir)


_NC_CACHE = {}


def _get_nc(B, S, D, H, KV, HD, HO, QC):
    key = (B, S, D, H, KV, HD, HO, QC)
    if key not in _NC_CACHE:
        _NC_CACHE[key] = build_nc(B, S, D, H, KV, HD, HO, QC)
    return _NC_CACHE[key]


def kernel(x, cos, sin, Wq, Wk, Wv, Wo, _sim=False):
    x = np.asarray(x, dtype=np.float32)
    cos = np.asarray(cos, dtype=np.float32)
    sin = np.asarray(sin, dtype=np.float32)
    Wq = np.asarray(Wq, dtype=np.float32)
    Wk = np.asarray(Wk, dtype=np.float32)
    Wv = np.asarray(Wv, dtype=np.float32)
    Wo = np.asarray(Wo, dtype=np.float32)

    B, S, D = x.shape
    HD = cos.shape[1]
    H = Wq.shape[1] // HD
    KV = Wk.shape[1] // HD
    HO = Wq.shape[1]
    R = B * S
    QC = R // CORES

    nc = _get_nc(B, S, D, H, KV, HD, HO, QC)
    in_maps = make_in_maps(x, cos, sin, Wq, Wk, Wv, Wo, QC)

    if _sim:
        from concourse import bass_interp
        sim = bass_interp.MultiCoreSim(nc, CORES)
        for c in range(CORES):
            for k, v in in_maps[c].items():
                sim.cores[c].tensor(k)[:] = v
        sim.simulate(check_with_hw=False)
        shards = [np.array(sim.cores[c].mem_tensor("out")) for c in range(CORES)]
    else:
        from concourse.bass_utils import run_bass_kernel_spmd
        trace = os.environ.get("KERNEL_TRACE", "0") == "1"
        res = None
        if trace:
            try:
                _install_profile_shim()
                tmpdir = os.environ.get("KERNEL_TMPDIR") or None
                res = run_bass_kernel_spmd(nc, in_maps,
                                           core_ids=list(range(CORES)),
                                           trace=True, tmpdir=tmpdir)
            except Exception as e:  # fall back to untraced run
                print(f"traced run failed ({type(e).__name__}: {e}); "
                      f"retrying untraced")
                res = None
        if res is None:
            res = run_bass_kernel_spmd(nc, in_maps,
                                       core_ids=list(range(CORES)),
                                       trace=False)
        if res.exec_time_ns is not None:
            print(f"HW exec time: {res.exec_time_ns} ns")
        shards = [res.results[c]["out"] for c in range(CORES)]

    return np.concatenate(shards, axis=0).reshape(B, S, D).astype(np.float32)
